# revision 9
# baseline (speedup 1.0000x reference)
"""LIF fully-connected neuron layer on 8 Trainium2 NeuronCores.

reference semantics (per sample b, hidden unit h):
    x[b,t,h] = sum_d input[b,t,d] * W[h,d] + bias[h]
    m_t   = mem_{t-1} + x_t
    spike = m_t > THRESH
    mem_t = m_t * (1-spike) * DECAY
    out[b,t,h] = spike

Strategy:
  - Data-parallel over batch: core c handles samples [8c, 8c+8).
  - Host pre-transposes input to [d, t, b] so matmul operands load naturally
    (contraction dim d on partitions) -- zero on-device transposes.
  - The fp32 matmul is approximated as 2 PE passes accumulated in one fp32
    PSUM group (scale 2^12 folded into power-of-2 operand scalings, undone
    by the ScalarE copy):
      pass 1 (fp16, 1 cyc/row):  (2^6 Whi16)^T (2^6 rhi16)
      pass 2 (fp8 DoubleRow, 2 k-tiles/instr):
          slot0  (2^13 Wlo8)^T (2^-1 rhi8)     # Wlo = W - Whi16
          slot1  (2^4  Whi8)^T (2^8  rlo8)     # rlo = r - rhi16
    The dropped lo*lo term and fp8 roundings leave x err ~1.3e-5 std
    (host-simulated: 10/4.2M spike flips, rel 3e-3; gate is 2e-2).
  - Matmul: N=512 moving columns (64 timesteps x 8 samples) per window,
    8 h-tiles x (8 fp16 + 8 DoubleRow) k-steps into 8 PSUM banks.
  - ScalarE copies PSUM->SBUF applying scale 2^-12 + per-partition bias.
  - Scan: one fused custom DVE op per timestep over [128, 64] lanes
    (lane = (h_tile, b), partition = h_lo):
        u' = m * (m <= TH),  m = u*DECAY + x_t
    u' is the pre-decay post-reset membrane; spikes are derived per 64-step
    window as (u' == 0) and DMA'd out as uint8.  (u'==0 with no spike
    requires the membrane to be exactly 0.0 -- measure-zero, verified
    empirically.)
  - Host reassembles [B, T, H] fp32 from the device uint8 layout.
"""

import numpy as np
import ml_dtypes

# ---- problem constants (hardcoded per contest contract) ----
B, T, D, H = 64, 512, 1024, 1024
N_CORES = 8
B_L = B // N_CORES            # 8 samples per core
P = 128                       # partitions
DT, HT = D // P, H // P       # 8 k-tiles, 8 h-tiles
WT = 64                       # timesteps per matmul window
NW = T // WT                  # 8 windows
NCOL = WT * B_L               # 512 moving columns per window
F = HT * B_L                  # 64 scan lanes in free dim
BLK = 64                      # timesteps per spike/output block (= WT)
NB = T // BLK                 # 8 output blocks

DECAY = 200.0 / 255.0
THRESH = 0.3

F16 = np.float16
FP8 = ml_dtypes.float8_e4m3

_CACHE = {}


def _register_lif_op():
    from concourse.dve_spec import Spec, Src0, Src1, C0, C1, lower
    from concourse.dve_ops import (
        DveOp, OPS, CUSTOM_DVE_SPECS, _SUB_OPCODE_FOR_NAME, _CUSTOM_DVE_ROW_BASE,
    )
    from concourse.dve_uop import DveOpSpec

    name = "LIF_STEP_ANT"
    for op in OPS:
        if op.name == name:
            return op

    m = Src0 * C0 + Src1
    body = (m <= C1) * m

    def ref(in0, in1, s0, s1, imm2):
        mm = (in0 * np.float32(s0) + in1).astype(np.float32)
        return (mm * (mm <= np.float32(s1))).astype(np.float32)

    spec = Spec(body=body, reference=ref)
    opcode = _CUSTOM_DVE_ROW_BASE + len(OPS)
    shas = {}
    for ver in ("v3", "v4"):
        uops = lower(spec, ver=ver)
        shas[ver] = DveOpSpec(name=name, opcode=opcode, uops=uops, rd1_en=True).sha(ver)
    op = DveOp(name, spec, subdim=False, uops_sha=shas)
    OPS.append(op)
    _SUB_OPCODE_FOR_NAME[name] = opcode
    CUSTOM_DVE_SPECS[name] = spec
    return op


def _build():
    if "nc" in _CACHE:
        return _CACHE["nc"]
    from contextlib import ExitStack
    import concourse.bacc as bacc
    import concourse.tile as tile
    from concourse import mybir

    lif_op = _register_lif_op()

    nc = bacc.Bacc("TRN2", target_bir_lowering=False, debug=False,
                   num_devices=N_CORES)
    f32 = mybir.dt.float32
    f16 = mybir.dt.float16
    f8 = mybir.dt.float8e4
    u8 = mybir.dt.uint8
    # weights sliced per k-tile so the first matmuls start after ~0.25 MiB
    wh_d = [nc.dram_tensor(f"wh{dt}", [P, H], f16, kind="ExternalInput").ap()
            for dt in range(DT)]
    wdr_d = [nc.dram_tensor(f"wdr{dt}", [P, 2 * H], f8, kind="ExternalInput").ap()
             for dt in range(DT)]
    rh_d = nc.dram_tensor("rh", [D, T * B_L], f16, kind="ExternalInput").ap()
    rdr_d = nc.dram_tensor("rdr", [D, 2 * T * B_L], f8, kind="ExternalInput").ap()
    bias_d = nc.dram_tensor("bias", [P, HT], f32, kind="ExternalInput").ap()
    out_d = nc.dram_tensor("out", [NB, P, BLK * F], u8, kind="ExternalOutput").ap()

    with tile.TileContext(nc) as tc, ExitStack() as ctx:
        const_pool = ctx.enter_context(tc.tile_pool(name="const", bufs=1))
        rhs_pool = ctx.enter_context(tc.tile_pool(name="rhs", bufs=3))
        xs_pool = ctx.enter_context(tc.tile_pool(name="xs", bufs=2))
        psum_pool = ctx.enter_context(tc.tile_pool(name="psum", bufs=1, space="PSUM"))
        spk_pool = ctx.enter_context(tc.tile_pool(name="spk", bufs=2))

        # --- constants (per-k-tile weight tiles; DMAs pipeline with matmuls).
        # Weights go out on the Scalar/Tensor engines' DMA queues so they run
        # concurrently with the Sync-queue input loads (one HW ring each).
        wh_s = [const_pool.tile([P, H], f16, name=f"wh{dt}") for dt in range(DT)]
        wdr_s = [const_pool.tile([P, 2 * H], f8, name=f"wdr{dt}") for dt in range(DT)]
        for dt in range(DT):
            nc.scalar.dma_start(wh_s[dt][:], wh_d[dt])
        for dt in range(DT):
            nc.gpsimd.dma_start(wdr_s[dt][:], wdr_d[dt])
        bias_s = const_pool.tile([P, HT], f32)
        nc.scalar.dma_start(bias_s[:], bias_d)

        # --- membrane ring: 128 slots of F lanes; slot t%128 = u after step t
        ring = const_pool.tile([P, 2 * BLK * F], f32)
        nc.vector.memset(ring[:, (2 * BLK - 1) * F:], 0.0)

        rh_r = rh_d.rearrange("(dt p) n -> p dt n", dt=DT)
        # host packs rdr as [d, w, two, n]: a window slice is contiguous per d
        rdr_r = rdr_d.rearrange("(dt p) (w twon) -> p dt w twon", dt=DT, w=NW)

        for w in range(NW):
            # load input^T window: fp16 hi [d_lo,(dt,n)] + fp8 pair [d_lo,(dt,two,n)]
            rh_t = rhs_pool.tile([P, DT * NCOL], f16)
            nc.sync.dma_start(
                rh_t[:].rearrange("p (dt n) -> p dt n", dt=DT),
                rh_r[:, :, w * NCOL:(w + 1) * NCOL],
            )
            rdr_t = rhs_pool.tile([P, DT * 2 * NCOL], f8)
            nc.sync.dma_start(
                rdr_t[:].rearrange("p (dt twon) -> p dt twon", dt=DT),
                rdr_r[:, :, w],
            )
            # matmul: 8 h-tiles x (8 fp16 + 8 DoubleRow) k-steps -> fp32 PSUM
            pt = [psum_pool.tile([P, NCOL], f32, tag=f"g{ht}", name=f"pt{ht}")
                  for ht in range(HT)]
            for ht in range(HT):
                acc = pt[ht][:]
                for dt in range(DT):
                    nc.tensor.matmul(
                        acc,
                        wh_s[dt][:, ht * P: ht * P + P],
                        rh_t[:, dt * NCOL:(dt + 1) * NCOL],
                        start=(dt == 0), stop=False,
                    )
                for dt in range(DT):
                    nc.tensor.matmul(
                        acc,
                        wdr_s[dt][:].rearrange("p (two h) -> p two h", two=2)
                            [:, :, ht * P: ht * P + P],
                        rdr_t[:].rearrange("p (dt two n) -> p dt two n",
                                           dt=DT, two=2)[:, dt],
                        start=False, stop=(dt == DT - 1),
                        perf_mode=mybir.MatmulPerfMode.DoubleRow,
                    )
            # PSUM -> SBUF undoing the 2^12 operand scaling, plus bias (ScalarE)
            xs = xs_pool.tile([P, HT * NCOL], f32)        # [p, (ht, t64, b8)]
            for ht in range(HT):
                nc.scalar.activation(
                    xs[:, ht * NCOL:(ht + 1) * NCOL],
                    pt[ht][:],
                    mybir.ActivationFunctionType.Identity,
                    bias=bias_s[:, ht:ht + 1],
                    scale=float(2.0 ** -12),
                )
            # scan: one fused DVE op per timestep
            xs_r = xs[:].rearrange("p (ht t b) -> p t ht b", ht=HT, t=WT, b=B_L)
            for tt in range(WT):
                t = w * WT + tt
                s_out = (t % (2 * BLK)) * F
                s_in = ((t - 1) % (2 * BLK)) * F
                nc.vector._custom_dve(
                    lif_op,
                    out=ring[:, s_out:s_out + F],
                    in0=ring[:, s_in:s_in + F],
                    in1=xs_r[:, tt],
                    s0=DECAY,
                    s1=THRESH,
                )
            # derive spikes for the finished 64-step window
            half = (w % 2) * BLK * F
            spk = spk_pool.tile([P, BLK * F], u8)
            nc.vector.tensor_scalar(
                out=spk[:], in0=ring[:, half:half + BLK * F],
                scalar1=0.0, scalar2=None, op0=mybir.AluOpType.is_equal,
            )
            nc.gpsimd.dma_start(out_d[w], spk[:])

    nc.compile()
    _CACHE["nc"] = nc
    return nc


def _prep_in_maps(input_data, W, b):
    """Host-side: transpose + fp16/fp8 operand split, one in_map per core."""
    input_data = np.asarray(input_data, dtype=np.float32)
    W = np.asarray(W, dtype=np.float32)
    b = np.asarray(b, dtype=np.float32)

    wt = np.ascontiguousarray(W.T)                       # [d, h] fp32
    wh16 = (wt * 2.0 ** 6).astype(F16)                   # 2^6 Whi16
    Wlo = wt - wh16.astype(np.float32) * 2.0 ** -6
    wlo8 = (Wlo * 2.0 ** 13).astype(FP8)
    whi8 = (wt * 2.0 ** 4).astype(FP8)
    wdr = np.stack([wlo8, whi8], axis=1)                 # [d, two, h]
    wh_maps = {f"wh{dt}": np.ascontiguousarray(wh16[dt * P:(dt + 1) * P])
               for dt in range(DT)}
    wdr_maps = {f"wdr{dt}": np.ascontiguousarray(
                    wdr[dt * P:(dt + 1) * P].reshape(P, 2 * H))
                for dt in range(DT)}
    bias = np.ascontiguousarray(b.reshape(HT, P).T)      # [h_lo, ht]

    in_maps = []
    for c in range(N_CORES):
        xc = input_data[c * B_L:(c + 1) * B_L]           # [8, T, D]
        xin = np.ascontiguousarray(xc.transpose(2, 1, 0)).reshape(D, T * B_L)
        rh16 = (xin * 2.0 ** 6).astype(F16)              # 2^6 rhi16
        rlo = xin - rh16.astype(np.float32) * 2.0 ** -6
        rhi8 = (xin * 2.0 ** -1).astype(FP8)
        rlo8 = (rlo * 2.0 ** 8).astype(FP8)
        rdr = np.ascontiguousarray(
            np.stack([rhi8.reshape(D, NW, NCOL), rlo8.reshape(D, NW, NCOL)],
                     axis=2).reshape(D, 2 * T * B_L))
        m = {"rh": rh16, "rdr": rdr, "bias": bias}
        m.update(wh_maps)
        m.update(wdr_maps)
        in_maps.append(m)
    return in_maps


def _post(res):
    """Host-side: reassemble [B, T, H] fp32 from per-core device layout."""
    outs = []
    for c in range(N_CORES):
        o = res.results[c]["out"]                        # [NB, P, BLK*F] u8
        o = o.reshape(NB, P, BLK, HT, B_L)               # [blk, h_lo, t, ht, b]
        o = o.transpose(4, 0, 2, 3, 1).reshape(B_L, T, H)
        outs.append(o)
    return np.ascontiguousarray(
        np.concatenate(outs, axis=0).astype(np.float32))


def kernel(input_data, W, b):
    from concourse.bass_utils import run_bass_kernel_spmd

    nc = _build()
    in_maps = _prep_in_maps(input_data, W, b)
    res = run_bass_kernel_spmd(nc, in_maps, core_ids=list(range(N_CORES)))
    return _post(res)
